# revision 8
# baseline (speedup 1.0000x reference)
"""Causal RoPE self-attention, distributed over 8 TRN2 NeuronCores.

Sharding: batch (2) x head-groups (4 heads each) -> 8 cores.
Each core computes, for its (batch b, head-group hg):
    q/k/v projections for its 4 heads (tensor-parallel column split),
    RoPE, causal attention, and the row-parallel slice of the output
    projection, producing a partial output partialT = WoS^T @ attnT
    of shape [E, S].  The host sums the 4 partials per batch and adds bo.

On-device layout notes:
  - activations live transposed: qT/kT are [head-dim, seq] so the
    score matmul sT[k, q] = K Q^T contracts over d on partitions (the
    two heads of a pair row-tile the PE array at K=64 each).
  - V tiles carry 64 all-ones columns (cols 64:128), so the PV matmul
    replicates the softmax denominator onto psum partitions 64:128 at
    zero extra cost (matmul time = moving columns only); normalization
    is then a lane-parallel reciprocal + multiply, no DMA broadcast.
  - x, all weights, qT/kT, exp'd scores, and V are bf16 (full-rate
    TensorEngine, FWL weight loads, half DMA); every accumulation is
    fp32 in PSUM, and the softmax/normalization math is fp32.
  - causal masking zeroes the exp'd diagonal strip with a gpsimd
    affine_select; the PV matmul is split around it so only the
    128-col diagonal strip waits on the mask.
  - x is DMA'd in 512-column slices and weights ahead of it, so the
    first projection matmuls start ~1us in; all other PE work is
    drip-fed between attention key-blocks (with flush-before-use
    prerequisites) so the PE never idles long enough for the HAM
    clock gate to throttle it to 1.2 GHz.
"""

import ml_dtypes
import numpy as np

import concourse.tile as tile
from concourse import bacc, mybir
from concourse.bass_utils import run_bass_kernel_spmd

F32 = mybir.dt.float32
BF16 = mybir.dt.bfloat16
AF = mybir.ActivationFunctionType

B, S, E = 2, 2048, 1024
H, D = 16, 64
HPG = 4                # heads per core
DH = HPG * D           # 256 head-dims per core
NE = E // 128          # 8 e-chunks
NST = S // 128         # 16 s-tiles / key blocks
NG = S // 512          # 4 column groups of 512
ROPE_BASE = 10000.0

_SWAP_MASK = [i ^ 1 for i in range(32)]


def build_nc():
    """Build + compile the per-core Bass graph (same graph on all 8 cores)."""
    nc = bacc.Bacc("TRN2", target_bir_lowering=False, debug=False, num_devices=8)

    def din(name, shape, dt=F32):
        return nc.dram_tensor(name, shape, dt, kind="ExternalInput").ap()

    xT = din("xT", [E, S], BF16)
    wqT = din("wqT", [E, DH], BF16)
    wkT = din("wkT", [E, DH], BF16)
    wvT = din("wvT", [E, DH], BF16)
    woST = din("woST", [DH, E], BF16)
    bq2 = din("bq2", [128, 2])
    bk2 = din("bk2", [128, 2])
    bvbc = din("bvbc", [128, DH])
    cos2 = din("cos2", [128, S], BF16)      # cosT duplicated on both halves
    sin2 = din("sin2", [128, S], BF16)      # signed sinT duplicated on both halves
    out = nc.dram_tensor("out", [E, S], F32, kind="ExternalOutput").ap()

    xT_r = xT.rearrange("(n p) s -> n p s", p=128)
    wq_r = wqT.rearrange("(n p) d -> n p d", p=128)
    wk_r = wkT.rearrange("(n p) d -> n p d", p=128)
    wv_r = wvT.rearrange("(n p) d -> n p d", p=128)
    wo_r = woST.rearrange("(n p) e -> n p e", p=128)
    out_r = out.rearrange("(n p) s -> n p s", p=128)

    with tile.TileContext(nc) as tc, nc.allow_low_precision(
            reason="bf16 matmul operands; fp32 PSUM accumulation throughout"):
        _emit(tc, nc, dict(
            xT_r=xT_r, wq_r=wq_r, wk_r=wk_r, wv_r=wv_r, wo_r=wo_r, out_r=out_r,
            bq2=bq2, bk2=bk2, bvbc=bvbc, cos2=cos2, sin2=sin2,
        ))
    nc.compile()
    return nc


def _emit(tc, nc, d):
    from contextlib import ExitStack
    ctx = ExitStack()
    with ctx:
        consts = ctx.enter_context(tc.tile_pool(name="consts", bufs=1))
        px = ctx.enter_context(tc.tile_pool(name="px", bufs=8))
        pwq = ctx.enter_context(tc.tile_pool(name="pwq", bufs=8))
        pwk = ctx.enter_context(tc.tile_pool(name="pwk", bufs=8))
        pwv = ctx.enter_context(tc.tile_pool(name="pwv", bufs=8))
        pwo = ctx.enter_context(tc.tile_pool(name="pwo", bufs=2))
        pqt = ctx.enter_context(tc.tile_pool(name="pqt", bufs=8))
        pkt = ctx.enter_context(tc.tile_pool(name="pkt", bufs=8))
        pv = ctx.enter_context(tc.tile_pool(name="pv", bufs=16))
        pat = ctx.enter_context(tc.tile_pool(name="pat", bufs=6))
        ptmp = ctx.enter_context(tc.tile_pool(name="ptmp", bufs=6))
        pe_ = ctx.enter_context(tc.tile_pool(name="pe", bufs=8))
        prec = ctx.enter_context(tc.tile_pool(name="prec", bufs=4))
        pout = ctx.enter_context(tc.tile_pool(name="pout", bufs=4))
        psc = ctx.enter_context(tc.tile_pool(name="psc", bufs=2, space="PSUM"))
        ppv = ctx.enter_context(tc.tile_pool(name="ppv", bufs=2, space="PSUM"))
        pbg = ctx.enter_context(tc.tile_pool(name="pbg", bufs=2, space="PSUM"))

        # ---- input DMAs, ordered by first use. x streams in 512-col
        # slices so the first k/q projections start as soon as the first
        # column group lands.
        wq_sb, wk_sb, wv_sb, wo_sb = [], [], [], []
        xt_sb = [px.tile([128, S], BF16, tag="xt", name=f"xt{e}")
                 for e in range(NE)]

        def dma_x_group(g):
            cols = slice(g * 512, (g + 1) * 512)
            for e in range(NE):
                nc.sync.dma_start(xt_sb[e][:, cols], d["xT_r"][e][:, cols])

        for e in range(NE):
            t = pwk.tile([128, DH], BF16, tag="wk")
            nc.sync.dma_start(t, d["wk_r"][e])
            wk_sb.append(t)
        dma_x_group(0)
        for e in range(NE):
            t = pwq.tile([128, DH], BF16, tag="wq")
            nc.sync.dma_start(t, d["wq_r"][e])
            wq_sb.append(t)
        cos2_sb = consts.tile([128, S], BF16)
        sin2_sb = consts.tile([128, S], BF16)
        nc.sync.dma_start(cos2_sb[:, 0:512], d["cos2"][:, 0:512])
        nc.sync.dma_start(sin2_sb[:, 0:512], d["sin2"][:, 0:512])
        bq2_sb = consts.tile([128, 2], F32)
        nc.sync.dma_start(bq2_sb, d["bq2"])
        bk2_sb = consts.tile([128, 2], F32)
        nc.sync.dma_start(bk2_sb, d["bk2"])
        for e in range(NE):
            t = pwv.tile([128, DH], BF16, tag="wv")
            nc.sync.dma_start(t, d["wv_r"][e])
            wv_sb.append(t)
        bvbc_sb = consts.tile([128, DH], F32)
        nc.sync.dma_start(bvbc_sb, d["bvbc"])
        dma_x_group(1)
        nc.sync.dma_start(cos2_sb[:, 512:1024], d["cos2"][:, 512:1024])
        nc.sync.dma_start(sin2_sb[:, 512:1024], d["sin2"][:, 512:1024])
        dma_x_group(2)
        dma_x_group(3)
        nc.sync.dma_start(cos2_sb[:, 1024:2048], d["cos2"][:, 1024:2048])
        nc.sync.dma_start(sin2_sb[:, 1024:2048], d["sin2"][:, 1024:2048])
        for p in range(2):
            t = pwo.tile([128, E], BF16, tag="wo")
            nc.sync.dma_start(t, d["wo_r"][p])
            wo_sb.append(t)

        qt_tiles, kt_tiles, at_tiles = {}, {}, {}
        v_sb = {}

        # ---- background unit machinery: a queue of (cost, fn) units,
        # drip-fed between attention key-blocks; flush indices force
        # prerequisites to be emitted before their consumers.
        bg_units = []
        bg_pos = [0]

        def bg_add(gen):
            bg_units.extend(gen)
            return len(bg_units)

        def bg_flush_until(idx):
            while bg_pos[0] < idx:
                bg_units[bg_pos[0]][1]()
                bg_pos[0] += 1

        def bg_take(budget):
            while budget > 0 and bg_pos[0] < len(bg_units):
                cost, fn = bg_units[bg_pos[0]]
                fn()
                bg_pos[0] += 1
                budget -= cost

        def emit_qk_unit(w_sb, bias_sb, dst_pool, dst_tag, tiles, p, g):
            """One k/q projection tile [128, 512] for (head-pair p, col
            group g): 8 accumulating matmuls + a RoPE tail."""
            ps = pbg.tile([128, 512], F32, tag="bg")
            cols = slice(g * 512, (g + 1) * 512)
            for e0 in range(0, NE, 2):
                def unit(e0=e0):
                    for e in (e0, e0 + 1):
                        nc.tensor.matmul(
                            ps,
                            w_sb[e][:, p * 128:(p + 1) * 128],
                            xt_sb[e][:, cols],
                            start=(e == 0), stop=(e == NE - 1),
                        )
                yield 0.43, unit
            def rope_tail():
                tq = ptmp.tile([128, 512], BF16, tag="tmpb")
                nc.vector.tensor_scalar_add(tq, ps, bias_sb[:, p:p + 1])
                tsh = ptmp.tile([128, 512], BF16, tag="tmpb")
                nc.vector.stream_shuffle(tsh, tq, _SWAP_MASK)
                nc.vector.tensor_mul(tsh, tsh, sin2_sb[:, cols])
                nc.vector.tensor_mul(tq, tq, cos2_sb[:, cols])
                qt = dst_pool.tile([128, 512], BF16, tag=dst_tag)
                nc.vector.tensor_add(qt, tq, tsh)
                tiles[(p, g)] = qt
            yield 0.25, rope_tail

        def emit_v_unit(st):
            def unit():
                psv = pbg.tile([128, 512], F32, tag="bg")
                for e in range(NE):
                    nc.tensor.matmul(
                        psv[:, 0:DH],
                        xt_sb[e][:, st * 128:(st + 1) * 128],
                        wv_sb[e],
                        start=(e == 0), stop=(e == NE - 1),
                    )
                # per head pair: head A = [vals | ones], head B = [ones | vals]
                # so PV psum puts A-vals/B-dens at partitions 0:64 and
                # A-dens/B-vals at 64:128 (all DVE ops stay base-aligned).
                vt = pv.tile([128, HPG, 128], BF16, tag="v")
                vt2 = vt.rearrange("p (pp two) c -> p pp two c", two=2)
                psv_r = psv[:, 0:DH].rearrange(
                    "p (pp two dd) -> p pp two dd", two=2, dd=64)
                bv_r = bvbc_sb.rearrange(
                    "p (pp two dd) -> p pp two dd", two=2, dd=64)
                nc.vector.memset(vt2[:, :, 0, 64:128], 1.0)
                nc.vector.memset(vt2[:, :, 1, 0:64], 1.0)
                nc.vector.tensor_add(
                    vt2[:, :, 0, 0:64], psv_r[:, :, 0, :], bv_r[:, :, 0, :])
                nc.vector.tensor_add(
                    vt2[:, :, 1, 64:128], psv_r[:, :, 1, :], bv_r[:, :, 1, :])
                v_sb[st] = vt
            yield 0.95, unit

        def emit_op_unit(j):
            """Output projection for q-slice j: 8 chunks of [128, 512]."""
            for c in range(NE):
                def unit(c=c):
                    pso = pbg.tile([128, 512], F32, tag="bg")
                    for p in range(2):
                        nc.tensor.matmul(
                            pso,
                            wo_sb[p][:, c * 128:(c + 1) * 128],
                            at_tiles[(p, j)],
                            start=(p == 0), stop=(p == 1),
                        )
                    stg = pout.tile([128, 512], F32, tag="stg")
                    if c % 2 == 0:
                        nc.vector.tensor_copy(stg, pso)
                    else:
                        nc.scalar.copy(stg, pso)
                    dst = d["out_r"][c][:, j * 512:(j + 1) * 512]
                    if j == NG - 1:
                        # tail: split across two queues to halve drain time
                        nc.sync.dma_start(dst[:, 0:256], stg[:, 0:256])
                        nc.sync.dma_start(dst[:, 256:512], stg[:, 256:512])
                    else:
                        nc.sync.dma_start(dst, stg)
                yield 0.6, unit

        def emit_attn(p, j, kb_flush):
            """Attention for head-pair p, query slice j (cols j*512...).
            kb_flush[kb] (optional) is a bg index to flush before that
            key-block's PV matmuls."""
            pvA = ppv.tile([128, 512], F32, tag="ppv")
            pvB = ppv.tile([128, 512], F32, tag="ppv")
            nkb = 4 * j + 4
            # start=True clears has_written for the WHOLE psum bank, so it
            # must only be set on the very first PV matmul per bank; fresh
            # elements are overwritten (not accumulated) by start=False
            # writes anyway, per-element.
            pv_started = [False, False]

            def pv_mm(idx, psum, vt, ets, cols, last):
                nc.tensor.matmul(
                    psum[:, cols], vt, ets,
                    start=not pv_started[idx], stop=last,
                )
                pv_started[idx] = True
            for kb in range(nkb):
                m = kb - 4 * j
                c0 = 128 * m if m > 0 else 0
                kt = kt_tiles[(p, kb // 4)]
                kcols = slice((kb % 4) * 128, (kb % 4) * 128 + 128)
                qt = qt_tiles[(p, j)]
                sc = psc.tile([128, 2, 512], F32, tag="sc")
                nc.tensor.matmul(
                    sc[:, 0, c0:512],
                    kt[0:64, kcols],
                    qt[0:64, c0:512],
                    start=True, stop=True, tile_position=(0, 0),
                )
                nc.tensor.matmul(
                    sc[:, 1, c0:512],
                    kt[64:128, kcols],
                    qt[64:128, c0:512],
                    start=True, stop=True, tile_position=(64, 0),
                )
                et = pe_.tile([128, 2, 512], BF16, tag="e")
                nc.scalar.activation(
                    et[:, :, c0:512], sc[:, :, c0:512], AF.Exp, scale=0.125)
                if kb in kb_flush:
                    bg_flush_until(kb_flush[kb])
                hA, hB = 2 * p, 2 * p + 1
                last = (kb == nkb - 1)
                if m >= 0:
                    # split PV around the causal mask: only the 128-col
                    # diagonal strip waits for the gpsimd affine_select
                    d1 = c0 + 128
                    if d1 < 512:
                        pv_mm(0, pvA, v_sb[kb][:, hA, :], et[:, 0, d1:512],
                              slice(d1, 512), False)
                        pv_mm(1, pvB, v_sb[kb][:, hB, :], et[:, 1, d1:512],
                              slice(d1, 512), False)
                    nc.gpsimd.affine_select(
                        out=et[:, :, c0:d1],
                        in_=et[:, :, c0:d1],
                        compare_op=mybir.AluOpType.is_ge,
                        fill=0.0,
                        base=0,
                        pattern=[[0, 2], [1, 128]],
                        channel_multiplier=-1,
                    )
                    pv_mm(0, pvA, v_sb[kb][:, hA, :], et[:, 0, c0:d1],
                          slice(c0, d1), last)
                    pv_mm(1, pvB, v_sb[kb][:, hB, :], et[:, 1, c0:d1],
                          slice(c0, d1), last)
                else:
                    pv_mm(0, pvA, v_sb[kb][:, hA, :], et[:, 0, 0:512],
                          slice(0, 512), last)
                    pv_mm(1, pvB, v_sb[kb][:, hB, :], et[:, 1, 0:512],
                          slice(0, 512), last)
                bg_take(0.45)
            # normalization: V's ones-columns replicated the denominator
            # into pvA rows 64:128 / pvB rows 0:64 at zero matmul cost.
            # Gather dens to base 0 (copy honors partition offsets), one
            # lane-parallel reciprocal, then base-aligned multiplies
            # (custom-DVE ops require base-0 operands; plain tensor ops
            # require equal bases).
            dw = prec.tile([128, 512], F32, tag="dw")
            nc.vector.tensor_copy(dw[0:64, :], pvA[64:128, :])
            nc.vector.tensor_copy(dw[64:128, :], pvB[0:64, :])
            rec = prec.tile([128, 512], F32, tag="rec")
            nc.vector.reciprocal_approx_fast(rec, dw)
            at = pat.tile([128, 512], BF16, tag="at")
            nc.vector.tensor_mul(at[0:64], pvA[0:64], rec[0:64])
            nc.vector.tensor_mul(at[64:128], pvB[64:128], rec[64:128])
            at_tiles[(p, j)] = at

        # ---- head: k00/q00 interleaved at e-chunk granularity (tracks
        # DMA arrival), then v0; everything else is background.
        ps_k = pbg.tile([128, 512], F32, tag="bg")
        ps_q = pbg.tile([128, 512], F32, tag="bg")
        for e in range(NE):
            nc.tensor.matmul(
                ps_k, wk_sb[e][:, 0:128], xt_sb[e][:, 0:512],
                start=(e == 0), stop=(e == NE - 1),
            )
            nc.tensor.matmul(
                ps_q, wq_sb[e][:, 0:128], xt_sb[e][:, 0:512],
                start=(e == 0), stop=(e == NE - 1),
            )

        def rope_head(ps, bias_sb, dst_pool, dst_tag, tiles, p, g):
            cols = slice(g * 512, (g + 1) * 512)
            tq = ptmp.tile([128, 512], BF16, tag="tmpb")
            nc.vector.tensor_scalar_add(tq, ps, bias_sb[:, p:p + 1])
            tsh = ptmp.tile([128, 512], BF16, tag="tmpb")
            nc.vector.stream_shuffle(tsh, tq, _SWAP_MASK)
            nc.vector.tensor_mul(tsh, tsh, sin2_sb[:, cols])
            nc.vector.tensor_mul(tq, tq, cos2_sb[:, cols])
            qt = dst_pool.tile([128, 512], BF16, tag=dst_tag)
            nc.vector.tensor_add(qt, tq, tsh)
            tiles[(p, g)] = qt

        rope_head(ps_k, bk2_sb, pkt, "kt", kt_tiles, 0, 0)
        rope_head(ps_q, bq2_sb, pqt, "qt", qt_tiles, 0, 0)
        for cost, fn in emit_v_unit(0):
            fn()

        # ---- background queue, ordered so flush indices are monotone.
        K, Q, V = emit_qk_unit, emit_qk_unit, emit_v_unit
        i_v1 = bg_add(V(1))
        i_v2 = bg_add(V(2))
        i_v3 = bg_add(V(3))
        i_q01 = bg_add(Q(wq_sb, bq2_sb, pqt, "qt", qt_tiles, 0, 1))
        i_k01 = bg_add(K(wk_sb, bk2_sb, pkt, "kt", kt_tiles, 0, 1))
        i_v4 = bg_add(V(4)); i_v5 = bg_add(V(5))
        i_v6 = bg_add(V(6)); i_v7 = bg_add(V(7))
        i_q10 = bg_add(Q(wq_sb, bq2_sb, pqt, "qt", qt_tiles, 1, 0))
        i_k10 = bg_add(K(wk_sb, bk2_sb, pkt, "kt", kt_tiles, 1, 0))
        i_q11 = bg_add(Q(wq_sb, bq2_sb, pqt, "qt", qt_tiles, 1, 1))
        i_k11 = bg_add(K(wk_sb, bk2_sb, pkt, "kt", kt_tiles, 1, 1))

        emit_attn(0, 0, {1: i_v1, 2: i_v2, 3: i_v3})
        bg_flush_until(i_q01)
        emit_attn(0, 1, {4: max(i_k01, i_v4), 5: i_v5, 6: i_v6, 7: i_v7})
        bg_flush_until(i_k10)
        emit_attn(1, 0, {})

        i_op0 = bg_add(emit_op_unit(0))
        i_q02 = bg_add(Q(wq_sb, bq2_sb, pqt, "qt", qt_tiles, 0, 2))
        i_k02 = bg_add(K(wk_sb, bk2_sb, pkt, "kt", kt_tiles, 0, 2))
        i_v8 = bg_add(V(8)); i_v9 = bg_add(V(9))
        i_v10 = bg_add(V(10)); i_v11 = bg_add(V(11))

        bg_flush_until(i_k11)
        emit_attn(1, 1, {})

        i_op1 = bg_add(emit_op_unit(1))
        i_q12 = bg_add(Q(wq_sb, bq2_sb, pqt, "qt", qt_tiles, 1, 2))
        i_k12 = bg_add(K(wk_sb, bk2_sb, pkt, "kt", kt_tiles, 1, 2))

        bg_flush_until(i_q02)
        emit_attn(0, 2, {8: max(i_k02, i_v8), 9: i_v9, 10: i_v10, 11: i_v11})

        i_q03 = bg_add(Q(wq_sb, bq2_sb, pqt, "qt", qt_tiles, 0, 3))
        i_k03 = bg_add(K(wk_sb, bk2_sb, pkt, "kt", kt_tiles, 0, 3))
        i_v12 = bg_add(V(12)); i_v13 = bg_add(V(13))
        i_v14 = bg_add(V(14)); i_v15 = bg_add(V(15))

        bg_flush_until(i_q12)
        emit_attn(1, 2, {8: i_k12})

        i_op2 = bg_add(emit_op_unit(2))
        i_q13 = bg_add(Q(wq_sb, bq2_sb, pqt, "qt", qt_tiles, 1, 3))
        i_k13 = bg_add(K(wk_sb, bk2_sb, pkt, "kt", kt_tiles, 1, 3))

        bg_flush_until(i_q03)
        emit_attn(0, 3, {12: max(i_k03, i_v12), 13: i_v13, 14: i_v14, 15: i_v15})

        bg_flush_until(i_q13)
        emit_attn(1, 3, {12: i_k13})

        bg_flush_until(len(bg_units))
        for cost, fn in emit_op_unit(3):
            fn()


def make_host_inputs(x, Wq, bq, Wk, bk, Wv, bv, Wo, bo):
    """Shard + pre-transpose inputs per core. Returns (in_maps, bo)."""
    x = np.asarray(x, np.float32)
    Wq, Wk, Wv, Wo = (np.asarray(w, np.float32) for w in (Wq, Wk, Wv, Wo))
    bq, bk, bv, bo = (np.asarray(b_, np.float32) for b_ in (bq, bk, bv, bo))

    # RoPE tables
    half = D // 2
    inv_freq = 1.0 / (ROPE_BASE ** (np.arange(half, dtype=np.float64) / half))
    pos = np.arange(S, dtype=np.float64)
    sinus = pos[:, None] * inv_freq[None, :]           # [S, 32]
    sin_full = np.repeat(np.sin(sinus), 2, axis=1)     # [S, 64] interleave-dup
    cos_full = np.repeat(np.cos(sinus), 2, axis=1)
    sgn = np.where(np.arange(D) % 2 == 0, -1.0, 1.0)
    cos2 = np.tile(cos_full.T, (2, 1)).astype(ml_dtypes.bfloat16)
    sin2 = np.tile((sin_full * sgn[None, :]).T, (2, 1)).astype(ml_dtypes.bfloat16)

    xT = [np.ascontiguousarray(x[b_].T) for b_ in range(B)]
    in_maps = []
    for c in range(8):
        b_, hg = c // 4, c % 4
        rows = slice(DH * hg, DH * hg + DH)
        bf = ml_dtypes.bfloat16
        in_maps.append({
            "xT": xT[b_].astype(bf),
            "wqT": np.ascontiguousarray(Wq[rows].T).astype(bf),
            "wkT": np.ascontiguousarray(Wk[rows].T).astype(bf),
            "wvT": np.ascontiguousarray(Wv[rows].T).astype(bf),
            "woST": np.ascontiguousarray(Wo[:, rows].T).astype(bf),
            "bq2": np.ascontiguousarray(bq[rows].reshape(2, 128).T),
            "bk2": np.ascontiguousarray(bk[rows].reshape(2, 128).T),
            "bvbc": np.tile(bv[rows][None, :], (128, 1)).astype(np.float32),
            "cos2": cos2,
            "sin2": sin2,
        })
    return in_maps, bo


_NC_CACHE = {}


def get_nc():
    if "nc" not in _NC_CACHE:
        _NC_CACHE["nc"] = build_nc()
    return _NC_CACHE["nc"]


def kernel(**inputs):
    in_maps, bo = make_host_inputs(**inputs)
    nc = get_nc()
    res = run_bass_kernel_spmd(nc, in_maps, core_ids=list(range(8)))
    out = np.zeros((B, S, E), np.float32)
    for c in range(8):
        out[c // 4] += res.results[c]["out"].T
    out += bo[None, None, :]
    return out


# revision 13
# speedup vs baseline: 1.0311x; 1.0311x over previous
"""Causal RoPE self-attention, distributed over 8 TRN2 NeuronCores.

Sharding: batch (2) x head-groups (4 heads each) -> 8 cores.
Each core computes, for its (batch b, head-group hg):
    q/k/v projections for its 4 heads (tensor-parallel column split),
    RoPE, causal attention, and the row-parallel slice of the output
    projection, producing a partial output partialT = WoS^T @ attnT
    of shape [E, S].  The host sums the 4 partials per batch and adds bo.

On-device layout notes:
  - activations live transposed: qT/kT are [head-dim, seq] so the
    score matmul sT[k, q] = K Q^T contracts over d on partitions (the
    two heads of a pair row-tile the PE array at K=64 each).
  - V tiles carry 64 all-ones columns (cols 64:128), so the PV matmul
    replicates the softmax denominator onto psum partitions 64:128 at
    zero extra cost (matmul time = moving columns only); normalization
    is then a lane-parallel reciprocal + multiply, no DMA broadcast.
  - x, all weights, qT/kT, exp'd scores, and V are bf16 (full-rate
    TensorEngine, FWL weight loads, half DMA); every accumulation is
    fp32 in PSUM, and the softmax/normalization math is fp32.
  - causal masking zeroes the exp'd diagonal strip with a gpsimd
    affine_select; the PV matmul is split around it so only the
    128-col diagonal strip waits on the mask.
  - x is DMA'd in 512-column slices and weights ahead of it, so the
    first projection matmuls start ~1us in; all other PE work is
    drip-fed between attention key-blocks (with flush-before-use
    prerequisites) so the PE never idles long enough for the HAM
    clock gate to throttle it to 1.2 GHz.
"""

import ml_dtypes
import numpy as np

import concourse.tile as tile
from concourse import bacc, mybir
from concourse.bass_utils import run_bass_kernel_spmd

F32 = mybir.dt.float32
BF16 = mybir.dt.bfloat16
AF = mybir.ActivationFunctionType

B, S, E = 2, 2048, 1024
H, D = 16, 64
HPG = 4                # heads per core
DH = HPG * D           # 256 head-dims per core
NE = E // 128          # 8 e-chunks
NST = S // 128         # 16 s-tiles / key blocks
NG = S // 512          # 4 column groups of 512
ROPE_BASE = 10000.0

_SWAP_MASK = [i ^ 1 for i in range(32)]


def build_nc():
    """Build + compile the per-core Bass graph (same graph on all 8 cores)."""
    nc = bacc.Bacc("TRN2", target_bir_lowering=False, debug=False, num_devices=8)

    def din(name, shape, dt=F32):
        return nc.dram_tensor(name, shape, dt, kind="ExternalInput").ap()

    # all inputs pre-packed on the host into p-major layouts so each loads
    # with ONE large dma_start (HWDGE triggers cost ~630ns of sequencer
    # time each and are FIFO per ring — few big transfers beat many small)
    xP = din("xP", [128, NE, S], BF16)
    wkP = din("wkP", [128, NE, DH], BF16)
    wqP = din("wqP", [128, NE, DH], BF16)
    wvP = din("wvP", [128, NE, DH], BF16)
    woP = din("woP", [128, 2, E], BF16)
    csin = din("csin", [128, 2, S], BF16)   # [cosT; signed sinT], dup halves
    bias = din("bias", [128, 4 + DH])       # bq2 | bk2 | bvbc
    out = nc.dram_tensor("out", [E, S], F32, kind="ExternalOutput").ap()

    out_r = out.rearrange("(n p) s -> n p s", p=128)

    with tile.TileContext(nc) as tc, nc.allow_low_precision(
            reason="bf16 matmul operands; fp32 PSUM accumulation throughout"):
        _emit(tc, nc, dict(
            xP=xP, wkP=wkP, wqP=wqP, wvP=wvP, woP=woP, csin=csin,
            bias=bias, out_r=out_r,
        ))
    nc.compile()
    return nc


def _emit(tc, nc, d):
    from contextlib import ExitStack
    ctx = ExitStack()
    with ctx:
        consts = ctx.enter_context(tc.tile_pool(name="consts", bufs=1))
        px = ctx.enter_context(tc.tile_pool(name="px", bufs=1))
        pwq = ctx.enter_context(tc.tile_pool(name="pwq", bufs=1))
        pwk = ctx.enter_context(tc.tile_pool(name="pwk", bufs=1))
        pwv = ctx.enter_context(tc.tile_pool(name="pwv", bufs=1))
        pwo = ctx.enter_context(tc.tile_pool(name="pwo", bufs=1))
        pqt = ctx.enter_context(tc.tile_pool(name="pqt", bufs=8))
        pkt = ctx.enter_context(tc.tile_pool(name="pkt", bufs=8))
        pv = ctx.enter_context(tc.tile_pool(name="pv", bufs=16))
        pat = ctx.enter_context(tc.tile_pool(name="pat", bufs=6))
        ptmp = ctx.enter_context(tc.tile_pool(name="ptmp", bufs=6))
        pe_ = ctx.enter_context(tc.tile_pool(name="pe", bufs=8))
        prec = ctx.enter_context(tc.tile_pool(name="prec", bufs=4))
        pout = ctx.enter_context(tc.tile_pool(name="pout", bufs=4))
        psc = ctx.enter_context(tc.tile_pool(name="psc", bufs=2, space="PSUM"))
        ppv = ctx.enter_context(tc.tile_pool(name="ppv", bufs=2, space="PSUM"))
        pbg = ctx.enter_context(tc.tile_pool(name="pbg", bufs=2, space="PSUM"))

        # ---- input DMAs: one big transfer per tensor group. The sync
        # ring carries weights; the scalar (Act) HWDGE ring carries x and
        # the RoPE tables concurrently, so the first k/q projections can
        # start ~4us in while the rest streams.
        xt_all = px.tile([128, NE, S], BF16)
        wk_all = pwk.tile([128, NE, DH], BF16)
        wq_all = pwq.tile([128, NE, DH], BF16)
        wv_all = pwv.tile([128, NE, DH], BF16)
        wo_all = pwo.tile([128, 2, E], BF16)
        csin_sb = consts.tile([128, 2, S], BF16)
        bias_sb = consts.tile([128, 4 + DH], F32)

        nc.sync.dma_start(wk_all, d["wkP"])
        nc.scalar.dma_start(xt_all[:, :, 0:512], d["xP"][:, :, 0:512])
        nc.sync.dma_start(wq_all, d["wqP"])
        nc.scalar.dma_start(csin_sb[:, :, 0:512], d["csin"][:, :, 0:512])
        nc.sync.dma_start(bias_sb, d["bias"])
        nc.sync.dma_start(wv_all, d["wvP"])
        nc.scalar.dma_start(xt_all[:, :, 512:1024], d["xP"][:, :, 512:1024])
        nc.scalar.dma_start(csin_sb[:, :, 512:2048], d["csin"][:, :, 512:2048])
        nc.sync.dma_start(xt_all[:, :, 1024:1536], d["xP"][:, :, 1024:1536])
        nc.sync.dma_start(xt_all[:, :, 1536:2048], d["xP"][:, :, 1536:2048])
        nc.sync.dma_start(wo_all, d["woP"])

        xt_sb = [xt_all[:, e, :] for e in range(NE)]
        wk_sb = [wk_all[:, e, :] for e in range(NE)]
        wq_sb = [wq_all[:, e, :] for e in range(NE)]
        wv_sb = [wv_all[:, e, :] for e in range(NE)]
        wo_sb = [wo_all[:, 0, :], wo_all[:, 1, :]]
        cos2_sb = csin_sb[:, 0, :]
        sin2_sb = csin_sb[:, 1, :]
        bq2_sb = bias_sb[:, 0:2]
        bk2_sb = bias_sb[:, 2:4]
        bvbc_sb = bias_sb[:, 4:4 + DH]

        qt_tiles, kt_tiles, at_tiles = {}, {}, {}
        v_sb = {}

        # ---- background unit machinery: a queue of (cost, fn) units,
        # drip-fed between attention key-blocks; flush indices force
        # prerequisites to be emitted before their consumers.
        bg_units = []
        bg_pos = [0]

        def bg_add(gen):
            bg_units.extend(gen)
            return len(bg_units)

        def bg_flush_until(idx):
            while bg_pos[0] < idx:
                bg_units[bg_pos[0]][1]()
                bg_pos[0] += 1

        def bg_take(budget):
            while budget > 0 and bg_pos[0] < len(bg_units):
                cost, fn = bg_units[bg_pos[0]]
                fn()
                bg_pos[0] += 1
                budget -= cost

        def emit_qk_unit(w_sb, bias_sb, dst_pool, dst_tag, tiles, p, g):
            """One k/q projection tile [128, 512] for (head-pair p, col
            group g): 8 accumulating matmuls + a RoPE tail."""
            ps = pbg.tile([128, 512], F32, tag="bg")
            cols = slice(g * 512, (g + 1) * 512)
            for e0 in range(0, NE, 2):
                def unit(e0=e0):
                    for e in (e0, e0 + 1):
                        nc.tensor.matmul(
                            ps,
                            w_sb[e][:, p * 128:(p + 1) * 128],
                            xt_sb[e][:, cols],
                            start=(e == 0), stop=(e == NE - 1),
                        )
                yield 0.43, unit
            def rope_tail():
                tq = ptmp.tile([128, 512], BF16, tag="tmpb")
                nc.vector.tensor_scalar_add(tq, ps, bias_sb[:, p:p + 1])
                tsh = ptmp.tile([128, 512], BF16, tag="tmpb")
                nc.vector.stream_shuffle(tsh, tq, _SWAP_MASK)
                nc.vector.tensor_mul(tsh, tsh, sin2_sb[:, cols])
                nc.vector.tensor_mul(tq, tq, cos2_sb[:, cols])
                qt = dst_pool.tile([128, 512], BF16, tag=dst_tag)
                nc.vector.tensor_add(qt, tq, tsh)
                tiles[(p, g)] = qt
            yield 0.25, rope_tail

        def emit_v_unit(st):
            def unit():
                psv = pbg.tile([128, 512], F32, tag="bg")
                for e in range(NE):
                    nc.tensor.matmul(
                        psv[:, 0:DH],
                        xt_sb[e][:, st * 128:(st + 1) * 128],
                        wv_sb[e],
                        start=(e == 0), stop=(e == NE - 1),
                    )
                # per head pair: head A = [vals | ones], head B = [ones | vals]
                # so PV psum puts A-vals/B-dens at partitions 0:64 and
                # A-dens/B-vals at 64:128 (all DVE ops stay base-aligned).
                vt = pv.tile([128, HPG, 128], BF16, tag="v")
                vt2 = vt.rearrange("p (pp two) c -> p pp two c", two=2)
                psv_r = psv[:, 0:DH].rearrange(
                    "p (pp two dd) -> p pp two dd", two=2, dd=64)
                bv_r = bvbc_sb.rearrange(
                    "p (pp two dd) -> p pp two dd", two=2, dd=64)
                nc.vector.memset(vt2[:, :, 0, 64:128], 1.0)
                nc.vector.memset(vt2[:, :, 1, 0:64], 1.0)
                nc.vector.tensor_add(
                    vt2[:, :, 0, 0:64], psv_r[:, :, 0, :], bv_r[:, :, 0, :])
                nc.vector.tensor_add(
                    vt2[:, :, 1, 64:128], psv_r[:, :, 1, :], bv_r[:, :, 1, :])
                v_sb[st] = vt
            yield 0.95, unit

        def emit_op_unit(j):
            """Output projection for q-slice j: 8 chunks of [128, 512]."""
            for c in range(NE):
                def unit(c=c):
                    pso = pbg.tile([128, 512], F32, tag="bg")
                    for p in range(2):
                        nc.tensor.matmul(
                            pso,
                            wo_sb[p][:, c * 128:(c + 1) * 128],
                            at_tiles[(p, j)],
                            start=(p == 0), stop=(p == 1),
                        )
                    stg = pout.tile([128, 512], F32, tag="stg")
                    if c % 2 == 0:
                        nc.vector.tensor_copy(stg, pso)
                    else:
                        nc.scalar.copy(stg, pso)
                    nc.sync.dma_start(
                        d["out_r"][c][:, j * 512:(j + 1) * 512], stg)
                yield 0.6, unit

        def emit_attn(p, j, kb_flush):
            """Attention for head-pair p, query slice j (cols j*512...).
            kb_flush[kb] (optional) is a bg index to flush before that
            key-block's PV matmuls."""
            pvA = ppv.tile([128, 512], F32, tag="ppv")
            pvB = ppv.tile([128, 512], F32, tag="ppv")
            nkb = 4 * j + 4
            # start=True clears has_written for the WHOLE psum bank, so it
            # must only be set on the very first PV matmul per bank; fresh
            # elements are overwritten (not accumulated) by start=False
            # writes anyway, per-element.
            pv_started = [False, False]

            def pv_mm(idx, psum, vt, ets, cols, last):
                nc.tensor.matmul(
                    psum[:, cols], vt, ets,
                    start=not pv_started[idx], stop=last,
                )
                pv_started[idx] = True
            for kb in range(nkb):
                m = kb - 4 * j
                c0 = 128 * m if m > 0 else 0
                kt = kt_tiles[(p, kb // 4)]
                kcols = slice((kb % 4) * 128, (kb % 4) * 128 + 128)
                qt = qt_tiles[(p, j)]
                sc = psc.tile([128, 2, 512], F32, tag="sc")
                nc.tensor.matmul(
                    sc[:, 0, c0:512],
                    kt[0:64, kcols],
                    qt[0:64, c0:512],
                    start=True, stop=True, tile_position=(0, 0),
                )
                nc.tensor.matmul(
                    sc[:, 1, c0:512],
                    kt[64:128, kcols],
                    qt[64:128, c0:512],
                    start=True, stop=True, tile_position=(64, 0),
                )
                et = pe_.tile([128, 2, 512], BF16, tag="e")
                nc.scalar.activation(
                    et[:, :, c0:512], sc[:, :, c0:512], AF.Exp, scale=0.125)
                if kb in kb_flush:
                    bg_flush_until(kb_flush[kb])
                hA, hB = 2 * p, 2 * p + 1
                last = (kb == nkb - 1)
                if m >= 0:
                    # split PV around the causal mask: only the 128-col
                    # diagonal strip waits for the gpsimd affine_select
                    d1 = c0 + 128
                    if d1 < 512:
                        pv_mm(0, pvA, v_sb[kb][:, hA, :], et[:, 0, d1:512],
                              slice(d1, 512), False)
                        pv_mm(1, pvB, v_sb[kb][:, hB, :], et[:, 1, d1:512],
                              slice(d1, 512), False)
                    nc.gpsimd.affine_select(
                        out=et[:, :, c0:d1],
                        in_=et[:, :, c0:d1],
                        compare_op=mybir.AluOpType.is_ge,
                        fill=0.0,
                        base=0,
                        pattern=[[0, 2], [1, 128]],
                        channel_multiplier=-1,
                    )
                    pv_mm(0, pvA, v_sb[kb][:, hA, :], et[:, 0, c0:d1],
                          slice(c0, d1), last)
                    pv_mm(1, pvB, v_sb[kb][:, hB, :], et[:, 1, c0:d1],
                          slice(c0, d1), last)
                else:
                    pv_mm(0, pvA, v_sb[kb][:, hA, :], et[:, 0, 0:512],
                          slice(0, 512), last)
                    pv_mm(1, pvB, v_sb[kb][:, hB, :], et[:, 1, 0:512],
                          slice(0, 512), last)
                bg_take(0.45)
            # normalization: V's ones-columns replicated the denominator
            # into pvA rows 64:128 / pvB rows 0:64 at zero matmul cost.
            # Gather dens to base 0 (copy honors partition offsets), one
            # lane-parallel reciprocal, then base-aligned multiplies
            # (custom-DVE ops require base-0 operands; plain tensor ops
            # require equal bases).
            dw = prec.tile([128, 512], F32, tag="dw")
            nc.vector.tensor_copy(dw[0:64, :], pvA[64:128, :])
            nc.vector.tensor_copy(dw[64:128, :], pvB[0:64, :])
            rec = prec.tile([128, 512], F32, tag="rec")
            nc.vector.reciprocal_approx_fast(rec, dw)
            at = pat.tile([128, 512], BF16, tag="at")
            nc.vector.tensor_mul(at[0:64], pvA[0:64], rec[0:64])
            nc.vector.tensor_mul(at[64:128], pvB[64:128], rec[64:128])
            at_tiles[(p, j)] = at

        # ---- head: k00/q00 interleaved at e-chunk granularity (tracks
        # DMA arrival), then v0; everything else is background.
        ps_k = pbg.tile([128, 512], F32, tag="bg")
        ps_q = pbg.tile([128, 512], F32, tag="bg")
        for e in range(NE):
            nc.tensor.matmul(
                ps_k, wk_sb[e][:, 0:128], xt_sb[e][:, 0:512],
                start=(e == 0), stop=(e == NE - 1),
            )
            nc.tensor.matmul(
                ps_q, wq_sb[e][:, 0:128], xt_sb[e][:, 0:512],
                start=(e == 0), stop=(e == NE - 1),
            )

        def rope_head(ps, bias_sb, dst_pool, dst_tag, tiles, p, g):
            cols = slice(g * 512, (g + 1) * 512)
            tq = ptmp.tile([128, 512], BF16, tag="tmpb")
            nc.vector.tensor_scalar_add(tq, ps, bias_sb[:, p:p + 1])
            tsh = ptmp.tile([128, 512], BF16, tag="tmpb")
            nc.vector.stream_shuffle(tsh, tq, _SWAP_MASK)
            nc.vector.tensor_mul(tsh, tsh, sin2_sb[:, cols])
            nc.vector.tensor_mul(tq, tq, cos2_sb[:, cols])
            qt = dst_pool.tile([128, 512], BF16, tag=dst_tag)
            nc.vector.tensor_add(qt, tq, tsh)
            tiles[(p, g)] = qt

        rope_head(ps_k, bk2_sb, pkt, "kt", kt_tiles, 0, 0)
        rope_head(ps_q, bq2_sb, pqt, "qt", qt_tiles, 0, 0)
        for cost, fn in emit_v_unit(0):
            fn()

        # ---- background queue, ordered so flush indices are monotone.
        K, Q, V = emit_qk_unit, emit_qk_unit, emit_v_unit
        i_v1 = bg_add(V(1))
        i_v2 = bg_add(V(2))
        i_v3 = bg_add(V(3))
        i_q01 = bg_add(Q(wq_sb, bq2_sb, pqt, "qt", qt_tiles, 0, 1))
        i_k01 = bg_add(K(wk_sb, bk2_sb, pkt, "kt", kt_tiles, 0, 1))
        i_v4 = bg_add(V(4)); i_v5 = bg_add(V(5))
        i_v6 = bg_add(V(6)); i_v7 = bg_add(V(7))
        i_q10 = bg_add(Q(wq_sb, bq2_sb, pqt, "qt", qt_tiles, 1, 0))
        i_k10 = bg_add(K(wk_sb, bk2_sb, pkt, "kt", kt_tiles, 1, 0))
        i_q11 = bg_add(Q(wq_sb, bq2_sb, pqt, "qt", qt_tiles, 1, 1))
        i_k11 = bg_add(K(wk_sb, bk2_sb, pkt, "kt", kt_tiles, 1, 1))

        emit_attn(0, 0, {1: i_v1, 2: i_v2, 3: i_v3})
        bg_flush_until(i_q01)
        emit_attn(0, 1, {4: max(i_k01, i_v4), 5: i_v5, 6: i_v6, 7: i_v7})
        bg_flush_until(i_k10)
        emit_attn(1, 0, {})

        i_op0 = bg_add(emit_op_unit(0))
        i_q02 = bg_add(Q(wq_sb, bq2_sb, pqt, "qt", qt_tiles, 0, 2))
        i_k02 = bg_add(K(wk_sb, bk2_sb, pkt, "kt", kt_tiles, 0, 2))
        i_v8 = bg_add(V(8)); i_v9 = bg_add(V(9))
        i_v10 = bg_add(V(10)); i_v11 = bg_add(V(11))

        bg_flush_until(i_k11)
        emit_attn(1, 1, {})

        i_op1 = bg_add(emit_op_unit(1))
        i_q12 = bg_add(Q(wq_sb, bq2_sb, pqt, "qt", qt_tiles, 1, 2))
        i_k12 = bg_add(K(wk_sb, bk2_sb, pkt, "kt", kt_tiles, 1, 2))

        bg_flush_until(i_q02)
        emit_attn(0, 2, {8: max(i_k02, i_v8), 9: i_v9, 10: i_v10, 11: i_v11})

        i_q03 = bg_add(Q(wq_sb, bq2_sb, pqt, "qt", qt_tiles, 0, 3))
        i_k03 = bg_add(K(wk_sb, bk2_sb, pkt, "kt", kt_tiles, 0, 3))
        i_v12 = bg_add(V(12)); i_v13 = bg_add(V(13))
        i_v14 = bg_add(V(14)); i_v15 = bg_add(V(15))

        bg_flush_until(i_q12)
        emit_attn(1, 2, {8: i_k12})

        i_op2 = bg_add(emit_op_unit(2))
        i_q13 = bg_add(Q(wq_sb, bq2_sb, pqt, "qt", qt_tiles, 1, 3))
        i_k13 = bg_add(K(wk_sb, bk2_sb, pkt, "kt", kt_tiles, 1, 3))

        bg_flush_until(i_q03)
        emit_attn(0, 3, {12: max(i_k03, i_v12), 13: i_v13, 14: i_v14, 15: i_v15})

        bg_flush_until(i_q13)
        emit_attn(1, 3, {12: i_k13})

        bg_flush_until(len(bg_units))
        for cost, fn in emit_op_unit(3):
            fn()


def make_host_inputs(x, Wq, bq, Wk, bk, Wv, bv, Wo, bo):
    """Shard + pre-transpose inputs per core. Returns (in_maps, bo)."""
    x = np.asarray(x, np.float32)
    Wq, Wk, Wv, Wo = (np.asarray(w, np.float32) for w in (Wq, Wk, Wv, Wo))
    bq, bk, bv, bo = (np.asarray(b_, np.float32) for b_ in (bq, bk, bv, bo))

    # RoPE tables
    half = D // 2
    inv_freq = 1.0 / (ROPE_BASE ** (np.arange(half, dtype=np.float64) / half))
    pos = np.arange(S, dtype=np.float64)
    sinus = pos[:, None] * inv_freq[None, :]           # [S, 32]
    sin_full = np.repeat(np.sin(sinus), 2, axis=1)     # [S, 64] interleave-dup
    cos_full = np.repeat(np.cos(sinus), 2, axis=1)
    sgn = np.where(np.arange(D) % 2 == 0, -1.0, 1.0)
    cos2 = np.tile(cos_full.T, (2, 1)).astype(ml_dtypes.bfloat16)
    sin2 = np.tile((sin_full * sgn[None, :]).T, (2, 1)).astype(ml_dtypes.bfloat16)

    bf = ml_dtypes.bfloat16

    def pmajor(a):
        """[NE*128, F] -> [128, NE, F] p-major packing."""
        n = a.shape[0] // 128
        return np.ascontiguousarray(
            a.reshape(n, 128, a.shape[1]).transpose(1, 0, 2))

    csin = np.ascontiguousarray(
        np.stack([cos2, sin2], axis=1))               # [128, 2, S] bf16
    xP = [pmajor(np.ascontiguousarray(x[b_].T)).astype(bf) for b_ in range(B)]
    in_maps = []
    for c in range(8):
        b_, hg = c // 4, c % 4
        rows = slice(DH * hg, DH * hg + DH)
        bias = np.concatenate([
            np.ascontiguousarray(bq[rows].reshape(2, 128).T),
            np.ascontiguousarray(bk[rows].reshape(2, 128).T),
            np.tile(bv[rows][None, :], (128, 1)),
        ], axis=1).astype(np.float32)                  # [128, 4+DH]
        in_maps.append({
            "xP": xP[b_],
            "wqP": pmajor(np.ascontiguousarray(Wq[rows].T)).astype(bf),
            "wkP": pmajor(np.ascontiguousarray(Wk[rows].T)).astype(bf),
            "wvP": pmajor(np.ascontiguousarray(Wv[rows].T)).astype(bf),
            "woP": pmajor(np.ascontiguousarray(Wo[:, rows].T)).astype(bf),
            "csin": csin,
            "bias": bias,
        })
    return in_maps, bo


_NC_CACHE = {}


def get_nc():
    if "nc" not in _NC_CACHE:
        _NC_CACHE["nc"] = build_nc()
    return _NC_CACHE["nc"]


def kernel(**inputs):
    in_maps, bo = make_host_inputs(**inputs)
    nc = get_nc()
    res = run_bass_kernel_spmd(nc, in_maps, core_ids=list(range(8)))
    out = np.zeros((B, S, E), np.float32)
    for c in range(8):
        out[c // 4] += res.results[c]["out"].T
    out += bo[None, None, :]
    return out


# revision 21
# speedup vs baseline: 1.1081x; 1.0747x over previous
"""Causal RoPE self-attention, distributed over 8 TRN2 NeuronCores.

Sharding: batch (2) x head-groups (4 heads each) -> 8 cores.
Each core computes, for its (batch b, head-group hg):
    q/k/v projections for its 4 heads (tensor-parallel column split),
    RoPE, causal attention, and the row-parallel slice of the output
    projection, producing a partial output partialT = WoS^T @ attnT
    of shape [E, S].  The host sums the 4 partials per batch and adds bo.

On-device layout notes:
  - activations live transposed: qT/kT are [head-dim, seq] so the
    score matmul sT[k, q] = K Q^T contracts over d on partitions (the
    two heads of a pair row-tile the PE array at K=64 each).
  - V tiles carry 64 all-ones columns (cols 64:128), so the PV matmul
    replicates the softmax denominator onto psum partitions 64:128 at
    zero extra cost (matmul time = moving columns only); normalization
    is then a lane-parallel reciprocal + multiply, no DMA broadcast.
  - x, all weights, qT/kT, exp'd scores, and V are bf16 (full-rate
    TensorEngine, FWL weight loads, half DMA); every accumulation is
    fp32 in PSUM, and the softmax/normalization math is fp32.
  - causal masking zeroes the exp'd diagonal strip with a gpsimd
    affine_select; the PV matmul is split around it so only the
    128-col diagonal strip waits on the mask.
  - x is DMA'd in 512-column slices and weights ahead of it, so the
    first projection matmuls start ~1us in; all other PE work is
    drip-fed between attention key-blocks (with flush-before-use
    prerequisites) so the PE never idles long enough for the HAM
    clock gate to throttle it to 1.2 GHz.
"""

import ml_dtypes
import numpy as np

import concourse.tile as tile
from concourse import bacc, mybir
from concourse.bass_utils import run_bass_kernel_spmd

F32 = mybir.dt.float32
BF16 = mybir.dt.bfloat16
AF = mybir.ActivationFunctionType

B, S, E = 2, 2048, 1024
H, D = 16, 64
HPG = 4                # heads per core
DH = HPG * D           # 256 head-dims per core
NE = E // 128          # 8 e-chunks
NST = S // 128         # 16 s-tiles / key blocks
NG = S // 512          # 4 column groups of 512
ROPE_BASE = 10000.0

_SWAP_MASK = [i ^ 1 for i in range(32)]


def build_nc():
    """Build + compile the per-core Bass graph (same graph on all 8 cores)."""
    nc = bacc.Bacc("TRN2", target_bir_lowering=False, debug=False, num_devices=8)

    def din(name, shape, dt=F32):
        return nc.dram_tensor(name, shape, dt, kind="ExternalInput").ap()

    # all inputs pre-packed on the host into p-major layouts so each loads
    # with ONE large dma_start (HWDGE triggers cost ~630ns of sequencer
    # time each and are FIFO per ring — few big transfers beat many small)
    xP = din("xP", [128, NE, S], BF16)
    wkP = din("wkP", [128, NE, DH], BF16)
    wqP = din("wqP", [128, NE, DH], BF16)
    wvP = din("wvP", [128, NE, DH], BF16)
    woP = din("woP", [128, 2, E], BF16)
    csin = din("csin", [128, 2, S], BF16)   # [cosT; signed sinT], dup halves
    bias = din("bias", [128, 4 + DH])       # bq2 | bk2 | bvbc
    out = nc.dram_tensor("out", [E, S], F32, kind="ExternalOutput").ap()

    out_r = out.rearrange("(n p) s -> n p s", p=128)

    with tile.TileContext(nc) as tc, nc.allow_low_precision(
            reason="bf16 matmul operands; fp32 PSUM accumulation throughout"):
        _emit(tc, nc, dict(
            xP=xP, wkP=wkP, wqP=wqP, wvP=wvP, woP=woP, csin=csin,
            bias=bias, out_r=out_r,
        ))
    nc.compile()
    return nc


def _emit(tc, nc, d):
    from contextlib import ExitStack
    ctx = ExitStack()
    with ctx:
        consts = ctx.enter_context(tc.tile_pool(name="consts", bufs=1))
        px = ctx.enter_context(tc.tile_pool(name="px", bufs=1))
        pwq = ctx.enter_context(tc.tile_pool(name="pwq", bufs=1))
        pwk = ctx.enter_context(tc.tile_pool(name="pwk", bufs=1))
        pwv = ctx.enter_context(tc.tile_pool(name="pwv", bufs=1))
        pwo = ctx.enter_context(tc.tile_pool(name="pwo", bufs=1))
        pqt = ctx.enter_context(tc.tile_pool(name="pqt", bufs=8))
        pkt = ctx.enter_context(tc.tile_pool(name="pkt", bufs=8))
        pv = ctx.enter_context(tc.tile_pool(name="pv", bufs=16))
        pat = ctx.enter_context(tc.tile_pool(name="pat", bufs=6))
        ptmp = ctx.enter_context(tc.tile_pool(name="ptmp", bufs=6))
        pe_ = ctx.enter_context(tc.tile_pool(name="pe", bufs=8))
        prec = ctx.enter_context(tc.tile_pool(name="prec", bufs=4))
        pout = ctx.enter_context(tc.tile_pool(name="pout", bufs=4))
        psc = ctx.enter_context(tc.tile_pool(name="psc", bufs=2, space="PSUM"))
        ppv = ctx.enter_context(tc.tile_pool(name="ppv", bufs=2, space="PSUM"))
        pbg = ctx.enter_context(tc.tile_pool(name="pbg", bufs=2, space="PSUM"))

        # ---- input DMAs: one big transfer per tensor group. The sync
        # ring carries weights; the scalar (Act) HWDGE ring carries x and
        # the RoPE tables concurrently, so the first k/q projections can
        # start ~4us in while the rest streams.
        xt_all = px.tile([128, NE, S], BF16)
        wk_all = pwk.tile([128, NE, DH], BF16)
        wq_all = pwq.tile([128, NE, DH], BF16)
        wv_all = pwv.tile([128, NE, DH], BF16)
        wo_all = pwo.tile([128, 2, E], BF16)
        csin_sb = consts.tile([128, 2, S], BF16)
        bias_sb = consts.tile([128, 4 + DH], F32)

        # sync ring: weights (p0 halves first — the head only needs p0);
        # act ring: x col-group 0 per e-chunk (the head's projection
        # matmuls chase chunk arrivals), then RoPE tables, then the rest.
        nc.sync.dma_start(wk_all[:, :, 0:128], d["wkP"][:, :, 0:128])
        for e in range(NE):
            nc.scalar.dma_start(xt_all[:, e, 0:512], d["xP"][:, e, 0:512])
        nc.sync.dma_start(wq_all[:, :, 0:128], d["wqP"][:, :, 0:128])
        nc.sync.dma_start(wk_all[:, :, 128:256], d["wkP"][:, :, 128:256])
        nc.sync.dma_start(wq_all[:, :, 128:256], d["wqP"][:, :, 128:256])
        nc.scalar.dma_start(csin_sb[:, :, 0:512], d["csin"][:, :, 0:512])
        nc.sync.dma_start(bias_sb, d["bias"])
        nc.sync.dma_start(wv_all, d["wvP"])
        nc.scalar.dma_start(csin_sb[:, :, 512:2048], d["csin"][:, :, 512:2048])
        nc.sync.dma_start(xt_all[:, :, 512:1024], d["xP"][:, :, 512:1024])
        nc.sync.dma_start(xt_all[:, :, 1024:1536], d["xP"][:, :, 1024:1536])
        nc.sync.dma_start(xt_all[:, :, 1536:2048], d["xP"][:, :, 1536:2048])
        nc.sync.dma_start(wo_all, d["woP"])

        xt_sb = [xt_all[:, e, :] for e in range(NE)]
        wk_sb = [wk_all[:, e, :] for e in range(NE)]
        wq_sb = [wq_all[:, e, :] for e in range(NE)]
        wv_sb = [wv_all[:, e, :] for e in range(NE)]
        wo_sb = [wo_all[:, 0, :], wo_all[:, 1, :]]
        cos2_sb = csin_sb[:, 0, :]
        sin2_sb = csin_sb[:, 1, :]
        bq2_sb = bias_sb[:, 0:2]
        bk2_sb = bias_sb[:, 2:4]
        bvbc_sb = bias_sb[:, 4:4 + DH]

        qt_tiles, kt_tiles, at_tiles = {}, {}, {}
        v_sb = {}

        # ---- background unit machinery: a queue of (cost, fn) units,
        # drip-fed between attention key-blocks; flush indices force
        # prerequisites to be emitted before their consumers.
        bg_units = []
        bg_pos = [0]

        def bg_add(gen):
            bg_units.extend(gen)
            return len(bg_units)

        def bg_flush_until(idx):
            while bg_pos[0] < idx:
                bg_units[bg_pos[0]][1]()
                bg_pos[0] += 1

        def bg_take(budget):
            while budget > 0 and bg_pos[0] < len(bg_units):
                cost, fn = bg_units[bg_pos[0]]
                fn()
                bg_pos[0] += 1
                budget -= cost

        def _rope(ps, bias_sb, dst_pool, dst_tag, tiles, p, g):
            """RoPE tail: the PSUM-reading bias-add + shuffle stay on DVE;
            the SBUF-only muls/add go to the (otherwise idle) GpSimd so
            DVE backlog never delays releasing the proj psum."""
            cols = slice(g * 512, (g + 1) * 512)
            tq = ptmp.tile([128, 512], BF16, tag="tmpb")
            nc.vector.tensor_scalar_add(tq, ps, bias_sb[:, p:p + 1])
            tsh = ptmp.tile([128, 512], BF16, tag="tmpb")
            nc.vector.stream_shuffle(tsh, tq, _SWAP_MASK)
            nc.gpsimd.tensor_mul(tsh, tsh, sin2_sb[:, cols])
            nc.gpsimd.tensor_mul(tq, tq, cos2_sb[:, cols])
            qt = dst_pool.tile([128, 512], BF16, tag=dst_tag)
            nc.gpsimd.tensor_add(qt, tq, tsh)
            tiles[(p, g)] = qt

        def emit_qk_unit(w_sb, bias_sb, dst_pool, dst_tag, tiles, p, g):
            """One k/q projection tile [128, 512] for (head-pair p, col
            group g): 8 accumulating matmuls + a RoPE tail."""
            ps = pbg.tile([128, 512], F32, tag="bg")
            cols = slice(g * 512, (g + 1) * 512)
            for e0 in range(0, NE, 2):
                def unit(e0=e0):
                    for e in (e0, e0 + 1):
                        nc.tensor.matmul(
                            ps,
                            w_sb[e][:, p * 128:(p + 1) * 128],
                            xt_sb[e][:, cols],
                            start=(e == 0), stop=(e == NE - 1),
                        )
                yield 0.43, unit
            def rope_tail():
                _rope(ps, bias_sb, dst_pool, dst_tag, tiles, p, g)
            yield 0.25, rope_tail

        def emit_v_unit(st):
            def unit():
                psv = pbg.tile([128, 512], F32, tag="bg")
                for e in range(NE):
                    nc.tensor.matmul(
                        psv[:, 0:DH],
                        xt_sb[e][:, st * 128:(st + 1) * 128],
                        wv_sb[e],
                        start=(e == 0), stop=(e == NE - 1),
                    )
                # per head pair: head A = [vals | ones], head B = [ones | vals]
                # so PV psum puts A-vals/B-dens at partitions 0:64 and
                # A-dens/B-vals at 64:128 (all DVE ops stay base-aligned).
                vt = pv.tile([128, HPG, 128], BF16, tag="v")
                vt2 = vt.rearrange("p (pp two) c -> p pp two c", two=2)
                psv_r = psv[:, 0:DH].rearrange(
                    "p (pp two dd) -> p pp two dd", two=2, dd=64)
                bv_r = bvbc_sb.rearrange(
                    "p (pp two dd) -> p pp two dd", two=2, dd=64)
                nc.gpsimd.memset(vt2[:, :, 0, 64:128], 1.0)
                nc.gpsimd.memset(vt2[:, :, 1, 0:64], 1.0)
                nc.vector.tensor_add(
                    vt2[:, :, 0, 0:64], psv_r[:, :, 0, :], bv_r[:, :, 0, :])
                nc.vector.tensor_add(
                    vt2[:, :, 1, 64:128], psv_r[:, :, 1, :], bv_r[:, :, 1, :])
                v_sb[st] = vt
            yield 0.95, unit

        def emit_op_unit(j):
            """Output projection for q-slice j: 8 chunks of [128, 512]."""
            for c in range(NE):
                def unit(c=c):
                    pso = pbg.tile([128, 512], F32, tag="bg")
                    for p in range(2):
                        nc.tensor.matmul(
                            pso,
                            wo_sb[p][:, c * 128:(c + 1) * 128],
                            at_tiles[(p, j)],
                            start=(p == 0), stop=(p == 1),
                        )
                    stg = pout.tile([128, 512], F32, tag="stg")
                    dst = d["out_r"][c][:, j * 512:(j + 1) * 512]
                    if c % 2 == 0:
                        nc.vector.tensor_copy(stg, pso)
                        nc.sync.dma_start(dst, stg)
                    elif j == NG - 1:
                        # tail: exp stream is done, scalar ring is free
                        nc.scalar.copy(stg, pso)
                        nc.scalar.dma_start(dst, stg)
                    else:
                        nc.scalar.copy(stg, pso)
                        nc.gpsimd.dma_start(dst, stg)
                yield 0.6, unit

        def emit_attn(p, j, kb_flush):
            """Attention for head-pair p, query slice j (cols j*512...).
            kb_flush[kb] (optional) is a bg index to flush before that
            key-block's PV matmuls."""
            pvA = ppv.tile([128, 512], F32, tag="ppv")
            pvB = ppv.tile([128, 512], F32, tag="ppv")
            nkb = 4 * j + 4
            # start=True clears has_written for the WHOLE psum bank, so it
            # must only be set on the very first PV matmul per bank; fresh
            # elements are overwritten (not accumulated) by start=False
            # writes anyway, per-element.
            pv_started = [False, False]

            def pv_mm(idx, psum, vt, ets, cols, last):
                nc.tensor.matmul(
                    psum[:, cols], vt, ets,
                    start=not pv_started[idx], stop=last,
                )
                pv_started[idx] = True
            for kb in range(nkb):
                m = kb - 4 * j
                c0 = 128 * m if m > 0 else 0
                kt = kt_tiles[(p, kb // 4)]
                kcols = slice((kb % 4) * 128, (kb % 4) * 128 + 128)
                qt = qt_tiles[(p, j)]
                sc = psc.tile([128, 2, 512], F32, tag="sc")
                nc.tensor.matmul(
                    sc[:, 0, c0:512],
                    kt[0:64, kcols],
                    qt[0:64, c0:512],
                    start=True, stop=True, tile_position=(0, 0),
                )
                nc.tensor.matmul(
                    sc[:, 1, c0:512],
                    kt[64:128, kcols],
                    qt[64:128, c0:512],
                    start=True, stop=True, tile_position=(64, 0),
                )
                et = pe_.tile([128, 2, 512], BF16, tag="e")
                nc.scalar.activation(
                    et[:, :, c0:512], sc[:, :, c0:512], AF.Exp, scale=0.125)
                if kb in kb_flush:
                    bg_flush_until(kb_flush[kb])
                hA, hB = 2 * p, 2 * p + 1
                last = (kb == nkb - 1)
                if m >= 0:
                    # split PV around the causal mask: only the 128-col
                    # diagonal strip waits for the gpsimd affine_select
                    d1 = c0 + 128
                    if d1 < 512:
                        pv_mm(0, pvA, v_sb[kb][:, hA, :], et[:, 0, d1:512],
                              slice(d1, 512), False)
                        pv_mm(1, pvB, v_sb[kb][:, hB, :], et[:, 1, d1:512],
                              slice(d1, 512), False)
                    nc.gpsimd.affine_select(
                        out=et[:, :, c0:d1],
                        in_=et[:, :, c0:d1],
                        compare_op=mybir.AluOpType.is_ge,
                        fill=0.0,
                        base=0,
                        pattern=[[0, 2], [1, 128]],
                        channel_multiplier=-1,
                    )
                    pv_mm(0, pvA, v_sb[kb][:, hA, :], et[:, 0, c0:d1],
                          slice(c0, d1), last)
                    pv_mm(1, pvB, v_sb[kb][:, hB, :], et[:, 1, c0:d1],
                          slice(c0, d1), last)
                else:
                    pv_mm(0, pvA, v_sb[kb][:, hA, :], et[:, 0, 0:512],
                          slice(0, 512), last)
                    pv_mm(1, pvB, v_sb[kb][:, hB, :], et[:, 1, 0:512],
                          slice(0, 512), last)
                bg_take(0.55)
            # normalization: V's ones-columns replicated the denominator
            # into pvA rows 64:128 / pvB rows 0:64 at zero matmul cost.
            # Gather dens to base 0 (copy honors partition offsets), one
            # lane-parallel reciprocal, then base-aligned multiplies
            # (custom-DVE ops require base-0 operands; plain tensor ops
            # require equal bases).
            dw = prec.tile([128, 512], F32, tag="dw")
            nc.vector.tensor_copy(dw[0:64, :], pvA[64:128, :])
            nc.vector.tensor_copy(dw[64:128, :], pvB[0:64, :])
            rec = prec.tile([128, 512], F32, tag="rec")
            nc.vector.reciprocal_approx_fast(rec, dw)
            at = pat.tile([128, 512], BF16, tag="at")
            nc.vector.tensor_mul(at[0:64], pvA[0:64], rec[0:64])
            nc.vector.tensor_mul(at[64:128], pvB[64:128], rec[64:128])
            at_tiles[(p, j)] = at

        # ---- head: k00/q00 interleaved at e-chunk granularity (tracks
        # DMA arrival), then v0; everything else is background.
        ps_k = pbg.tile([128, 512], F32, tag="bg")
        ps_q = pbg.tile([128, 512], F32, tag="bg")
        for e in range(NE):
            nc.tensor.matmul(
                ps_k, wk_sb[e][:, 0:128], xt_sb[e][:, 0:512],
                start=(e == 0), stop=(e == NE - 1),
            )
            nc.tensor.matmul(
                ps_q, wq_sb[e][:, 0:128], xt_sb[e][:, 0:512],
                start=(e == 0), stop=(e == NE - 1),
            )

        def rope_head(ps, bias_sb, dst_pool, dst_tag, tiles, p, g):
            cols = slice(g * 512, (g + 1) * 512)
            tq = ptmp.tile([128, 512], BF16, tag="tmpb")
            nc.vector.tensor_scalar_add(tq, ps, bias_sb[:, p:p + 1])
            tsh = ptmp.tile([128, 512], BF16, tag="tmpb")
            nc.vector.stream_shuffle(tsh, tq, _SWAP_MASK)
            nc.vector.tensor_mul(tsh, tsh, sin2_sb[:, cols])
            nc.vector.tensor_mul(tq, tq, cos2_sb[:, cols])
            qt = dst_pool.tile([128, 512], BF16, tag=dst_tag)
            nc.vector.tensor_add(qt, tq, tsh)
            tiles[(p, g)] = qt

        rope_head(ps_k, bk2_sb, pkt, "kt", kt_tiles, 0, 0)
        rope_head(ps_q, bq2_sb, pqt, "qt", qt_tiles, 0, 0)
        for cost, fn in emit_v_unit(0):
            fn()

        # ---- background queue in deadline order (each phase's q/k tiles
        # sit ahead of later-deadline V and op units, so the per-kb drip
        # finishes prerequisites before their phase starts and flush
        # bursts at transitions stay tiny).
        def Q(p, g):
            return bg_add(emit_qk_unit(wq_sb, bq2_sb, pqt, "qt", qt_tiles, p, g))

        def K(p, g):
            return bg_add(emit_qk_unit(wk_sb, bk2_sb, pkt, "kt", kt_tiles, p, g))

        def V(st):
            return bg_add(emit_v_unit(st))

        i_v1 = V(1); i_v2 = V(2); i_v3 = V(3)
        i_q10 = Q(1, 0); i_k10 = K(1, 0)
        i_q01 = Q(0, 1); i_k01 = K(0, 1)
        i_v4 = V(4); i_v5 = V(5); i_v6 = V(6); i_v7 = V(7)
        i_q11 = Q(1, 1); i_k11 = K(1, 1)

        emit_attn(0, 0, {1: i_v1, 2: i_v2, 3: i_v3})
        bg_flush_until(i_k10)
        emit_attn(1, 0, {})

        i_q02 = Q(0, 2); i_k02 = K(0, 2)
        i_v8 = V(8); i_v9 = V(9); i_v10 = V(10); i_v11 = V(11)

        bg_flush_until(i_q01)
        emit_attn(0, 1, {4: max(i_k01, i_v4), 5: i_v5, 6: i_v6, 7: i_v7})

        i_q12 = Q(1, 2); i_k12 = K(1, 2)
        i_op0 = bg_add(emit_op_unit(0))

        bg_flush_until(i_q11)
        emit_attn(1, 1, {4: i_k11})

        i_q03 = Q(0, 3); i_k03 = K(0, 3)
        i_v12 = V(12); i_v13 = V(13); i_v14 = V(14); i_v15 = V(15)

        bg_flush_until(i_q02)
        emit_attn(0, 2, {8: max(i_k02, i_v8), 9: i_v9, 10: i_v10, 11: i_v11})

        i_q13 = Q(1, 3); i_k13 = K(1, 3)
        i_op1 = bg_add(emit_op_unit(1))

        bg_flush_until(i_q12)
        emit_attn(1, 2, {8: i_k12})

        i_op2 = bg_add(emit_op_unit(2))

        bg_flush_until(i_q03)
        emit_attn(0, 3, {12: max(i_k03, i_v12), 13: i_v13, 14: i_v14, 15: i_v15})

        bg_flush_until(i_q13)
        emit_attn(1, 3, {12: i_k13})

        bg_flush_until(len(bg_units))
        for cost, fn in emit_op_unit(3):
            fn()


def make_host_inputs(x, Wq, bq, Wk, bk, Wv, bv, Wo, bo):
    """Shard + pre-transpose inputs per core. Returns (in_maps, bo)."""
    x = np.asarray(x, np.float32)
    Wq, Wk, Wv, Wo = (np.asarray(w, np.float32) for w in (Wq, Wk, Wv, Wo))
    bq, bk, bv, bo = (np.asarray(b_, np.float32) for b_ in (bq, bk, bv, bo))

    # RoPE tables
    half = D // 2
    inv_freq = 1.0 / (ROPE_BASE ** (np.arange(half, dtype=np.float64) / half))
    pos = np.arange(S, dtype=np.float64)
    sinus = pos[:, None] * inv_freq[None, :]           # [S, 32]
    sin_full = np.repeat(np.sin(sinus), 2, axis=1)     # [S, 64] interleave-dup
    cos_full = np.repeat(np.cos(sinus), 2, axis=1)
    sgn = np.where(np.arange(D) % 2 == 0, -1.0, 1.0)
    cos2 = np.tile(cos_full.T, (2, 1)).astype(ml_dtypes.bfloat16)
    sin2 = np.tile((sin_full * sgn[None, :]).T, (2, 1)).astype(ml_dtypes.bfloat16)

    bf = ml_dtypes.bfloat16

    def pmajor(a):
        """[NE*128, F] -> [128, NE, F] p-major packing."""
        n = a.shape[0] // 128
        return np.ascontiguousarray(
            a.reshape(n, 128, a.shape[1]).transpose(1, 0, 2))

    csin = np.ascontiguousarray(
        np.stack([cos2, sin2], axis=1))               # [128, 2, S] bf16
    xP = [pmajor(np.ascontiguousarray(x[b_].T)).astype(bf) for b_ in range(B)]
    in_maps = []
    for c in range(8):
        b_, hg = c // 4, c % 4
        rows = slice(DH * hg, DH * hg + DH)
        bias = np.concatenate([
            np.ascontiguousarray(bq[rows].reshape(2, 128).T),
            np.ascontiguousarray(bk[rows].reshape(2, 128).T),
            np.tile(bv[rows][None, :], (128, 1)),
        ], axis=1).astype(np.float32)                  # [128, 4+DH]
        in_maps.append({
            "xP": xP[b_],
            "wqP": pmajor(np.ascontiguousarray(Wq[rows].T)).astype(bf),
            "wkP": pmajor(np.ascontiguousarray(Wk[rows].T)).astype(bf),
            "wvP": pmajor(np.ascontiguousarray(Wv[rows].T)).astype(bf),
            "woP": pmajor(np.ascontiguousarray(Wo[:, rows].T)).astype(bf),
            "csin": csin,
            "bias": bias,
        })
    return in_maps, bo


_NC_CACHE = {}


def get_nc():
    if "nc" not in _NC_CACHE:
        _NC_CACHE["nc"] = build_nc()
    return _NC_CACHE["nc"]


def kernel(**inputs):
    in_maps, bo = make_host_inputs(**inputs)
    nc = get_nc()
    res = run_bass_kernel_spmd(nc, in_maps, core_ids=list(range(8)))
    out = np.zeros((B, S, E), np.float32)
    for c in range(8):
        out[c // 4] += res.results[c]["out"].T
    out += bo[None, None, :]
    return out
